# revision 2
# baseline (speedup 1.0000x reference)
"""Trainium2 Bass kernel v2 — PE column-pair tiling + bias-matmul removal.

vs v1: the 128x128 PE array was half-idle (batch M=64 occupies array cols
0-63). v2 splits every K-tile group into two halves that run CONCURRENTLY
in array col-groups 0-63 / 64-127 (tile_position via out base partition),
writing partials to PSUM partitions 0:64 / 64:128; one in-place DVE add
merges them. Projection biases move into the ACT activation's per-partition
bias operand; the d1/conductor LSTM biases ride K=64/K=1 matmuls paired
into otherwise-idle column groups.

Sharding: 8-way model parallel over the 4096 gate dim (512/core), batch
replicated; h/note exchanged per step via remote_dma_broadcast.
"""

import os
import sys

for _p in ("/opt/trn_rl_repo", "/root/.axon_site/_ro/trn_rl_repo"):
    if os.path.isdir(_p) and _p not in sys.path:
        sys.path.insert(0, _p)
        break

import numpy as np

from concourse import bass, mybir, bacc

F32 = mybir.dt.float32
F32R = mybir.dt.float32r

NC = 8           # cores
B = 64           # batch
H = 1024         # decoder hidden
HC = 1024        # conductor hidden
LATENT = 512
INPUT = 389
INPUT_PAD = 512
COND_OUT = 512
GSL = 512        # per-core gate slice (4*H/NC)
KT_H = 8         # K tiles of 128 over H
KT_L = 4         # K tiles of 128 over LATENT
SL = 64          # slot width (columns) in gathered buffers

RD = [(0, k) for k in range(NC)]


def build(nsub, nnotes, full_out=True):
    """Build the SPMD Bass program. nsub conductor steps, nsub*nnotes decoder steps."""
    CT = nsub
    T = nsub * nnotes
    TOUT = T if full_out else 1
    nc = bacc.Bacc(num_devices=NC)

    # ---------------- DRAM parameters (per-core data) ----------------
    dp = nc.declare_dram_parameter
    latT_d = dp("latT", [128, KT_L * SL], F32R, isOutput=False)
    h0T_d = dp("h0T", [nsub, 128, 2 * KT_H * SL], F32R, isOutput=False)
    c0s_d = dp("c0s", [B, nsub * 2 * 128], F32, isOutput=False)
    wx0_d = dp("wx0", [64, 8 * GSL], F32R, isOutput=False)
    wh0_d = dp("wh0", [128, 8 * GSL], F32R, isOutput=False)
    wx1_d = dp("wx1", [128, 8 * GSL], F32R, isOutput=False)
    wh1_d = dp("wh1", [128, 8 * GSL], F32R, isOutput=False)
    wdoT_d = dp("wdoT", [128, 8 * SL], F32R, isOutput=False)
    wemb_d = dp("wemb", [64, 8 * GSL], F32R, isOutput=False)
    wxc0_d = dp("wxc0", [128, KT_L * GSL], F32R, isOutput=False)
    whc0_d = dp("whc0", [128, 8 * GSL], F32R, isOutput=False)
    wxc1_d = dp("wxc1", [128, 8 * GSL], F32R, isOutput=False)
    whc1_d = dp("whc1", [128, 8 * GSL], F32R, isOutput=False)
    wcoT_d = dp("wcoT", [128, 8 * SL], F32R, isOutput=False)
    b0_d = dp("b0r", [1, GSL], F32R, isOutput=False)        # bih+bhh d0 (g x2)
    b1rep_d = dp("b1rep", [64, GSL], F32R, isOutput=False)  # b1 on all 64 rows
    bdoc_d = dp("bdoc", [64, 1], F32, isOutput=False)       # proj bias column
    bc0_d = dp("bc0r", [1, GSL], F32R, isOutput=False)
    bc1_d = dp("bc1r", [1, GSL], F32R, isOutput=False)
    bcoc_d = dp("bcoc", [64, 1], F32, isOutput=False)       # emb bias column
    ones_d = dp("onesr", [1, SL], F32R, isOutput=False)
    id64_d = dp("id64", [64, 64], F32R, isOutput=False)
    idT_d = dp("idT", [128, 128], F32R, isOutput=False)
    out_d = dp("out", [TOUT, 64, 64], F32, isOutput=True)
    est_d = nc.dram_tensor("est", [nsub, B, GSL], F32R)

    import contextlib
    with contextlib.ExitStack() as ctx:
        e = ctx.enter_context
        sb = lambda name, shape, dt=F32: e(nc.sbuf_tensor(name, shape, dt))
        ps = lambda name, shape: e(nc.psum_tensor(name, shape, F32))
        sem = lambda name: e(nc.semaphore(name))

        # weights / constants in SBUF
        LAT = sb("LAT", [128, KT_L * SL], F32R)
        H0T = sb("H0T", [128, 2 * (2 * KT_H * SL)], F32R)   # ping-pong per subseq
        C0SUB = sb("C0SUB", [B, 2 * 256])
        WX0 = sb("WX0", [64, 8 * GSL], F32R)
        WH0 = sb("WH0", [128, 8 * GSL], F32R)
        WX1 = sb("WX1", [128, 8 * GSL], F32R)
        WH1 = sb("WH1", [128, 8 * GSL], F32R)
        WDOT = sb("WDOT", [128, 8 * SL], F32R)
        WEMB = sb("WEMB", [64, 8 * GSL], F32R)
        WXC0 = sb("WXC0", [128, KT_L * GSL], F32R)
        WHC0 = sb("WHC0", [128, 8 * GSL], F32R)
        WXC1 = sb("WXC1", [128, 8 * GSL], F32R)
        WHC1 = sb("WHC1", [128, 8 * GSL], F32R)
        WCOT = sb("WCOT", [128, 8 * SL], F32R)
        B0 = sb("B0", [1, GSL], F32R)
        B1REP = sb("B1REP", [64, GSL], F32R)
        BDOC = sb("BDOC", [64, 1])
        BC0 = sb("BC0", [1, GSL], F32R)
        BC1 = sb("BC1", [1, GSL], F32R)
        BCOC = sb("BCOC", [64, 1])
        ONES = sb("ONES", [1, SL], F32R)
        ID64 = sb("ID64", [64, 64], F32R)
        IDT = sb("IDT", [128, 128], F32R)
        EBUF = sb("EBUF", [B, 2 * GSL], F32R)
        EDEC = sb("EDEC", [B, 2 * GSL], F32R)

        # gathered state buffers (ping-pong x2)
        HD0 = [sb(f"HD0_{i}", [128, NC * SL], F32R) for i in range(2)]
        HD1 = [sb(f"HD1_{i}", [128, NC * SL], F32R) for i in range(2)]
        NT = [sb(f"NT_{i}", [128, NC * SL], F32R) for i in range(2)]
        HC0 = [sb(f"HC0_{i}", [128, NC * SL], F32R) for i in range(2)]
        HC1 = [sb(f"HC1_{i}", [128, NC * SL], F32R) for i in range(2)]
        EMBT = [sb(f"EMBT_{i}", [128, NC * SL], F32R) for i in range(2)]

        # staging for outgoing tiles
        HSTG0 = [sb(f"HSTG0_{i}", [128, SL], F32R) for i in range(2)]
        HSTG1 = [sb(f"HSTG1_{i}", [128, SL], F32R) for i in range(2)]
        SNT = [sb(f"SNT_{i}", [128, SL], F32R) for i in range(2)]
        SEM_ = [sb(f"SEM_{i}", [128, SL], F32R) for i in range(2)]

        # activation work tiles (layer 0/1; conductor reuses the same)
        S0 = sb("S0", [B, GSL])
        S1 = sb("S1", [B, GSL])
        CC0 = sb("CC0", [B, 256])   # [g' | c] layer0 (decoder)
        CC1 = sb("CC1", [B, 256])
        CCC0 = sb("CCC0", [B, 256])  # conductor cell states
        CCC1 = sb("CCC1", [B, 256])
        TMP0 = sb("TMP0", [B, 256])
        TMP1 = sb("TMP1", [B, 256])
        TT0 = sb("TT0", [B, 128])
        TT1 = sb("TT1", [B, 128])
        HT0 = sb("HT0", [B, 128], F32R)   # h tiles (pre-transpose)
        HT1 = sb("HT1", [B, 128], F32R)

        # psum: [128, .]: partitions 0:64 = half A, 64:128 = half B
        psd0 = [ps(f"psd0_{i}", [128, GSL // 2]) for i in range(2)]
        psd1 = [ps(f"psd1_{i}", [128, GSL // 2]) for i in range(2)]
        pspr = ps("pspr", [128, 64])
        pstr0 = ps("pstr0", [128, 64])
        pstr1 = ps("pstr1", [128, 64])
        psem = ps("psem", [128, GSL // 2])

        # semaphores
        dw = sem("dw"); dh = sem("dh"); gi = sem("gi")
        do = [sem("doa"), sem("dob")]
        de = [sem("dea"), sem("deb")]; ep = [sem("epa"), sem("epb")]
        pe_s = sem("pe_s"); act_s = sem("act_s"); dve_s = sem("dve_s")
        r_h0 = sem("r_h0"); r_h1 = sem("r_h1"); r_nt = sem("r_nt"); r_em = sem("r_em")
        l_h0 = [sem("l_h0a"), sem("l_h0b")]; l_h1 = [sem("l_h1a"), sem("l_h1b")]
        l_nt = [sem("l_nta"), sem("l_ntb")]; l_em = [sem("l_ema"), sem("l_emb")]
        prep = sem("prep")

        N_MEMSET = 12  # NT/SNT/SEM_/EMBT x2, HC0[1], HC1[1], CCC0/1
        N_WLOAD = 21   # dma_start count on sync at init (x16 each)

        # ---- sem threshold helpers (single source of truth) ----
        def pe_c(ct, k):      # conductor: d0c=1, tr0=2, d1c=3, tr1=4, em=5, E=6
            return 6 * ct + k

        def pe_d(t, k):       # decoder: d0=1, tr0=2, d1=3, tr1=4, pr=5
            return 6 * CT + 5 * t + k

        # ACT per step: sigH0=1 sigL0=2 tanh0=3 sigH1=4 sigL1=5 tanh1=6 tanhP=7
        def act_c(ct, k):
            return 7 * ct + k

        def act_d(t, k):
            return 7 * CT + 7 * t + k

        # DVE per step: g'0=1 ti=2 tf=3 cn=4 h0=5 cp0=6
        #               g'1=7 ti=8 tf=9 cn=10 h1=11 cp1=12
        #               [conductor additionally: E_lo=13 E_hi=14]
        def dve_c(ct, k):
            return 14 * ct + k

        def dve_d(t, k):
            return 14 * CT + 12 * t + k

        def snd_c(ct):        # sends of parity-(ct%2) staging strictly before conductor step ct
            p = ct % 2
            return (ct - p) // 2

        def snd_d(t):         # conductor sends of this parity + decoder sends before t
            p = t % 2
            return (CT - p + 1) // 2 + (t - p) // 2

        with nc.Block() as block:

            # ================= SYNC: DMAs =================
            @block.sync
            def _(sy):
                loads = [
                    (LAT, latT_d),
                    (WX0, wx0_d), (WH0, wh0_d), (WX1, wx1_d), (WH1, wh1_d),
                    (WDOT, wdoT_d), (WEMB, wemb_d),
                    (WXC0, wxc0_d), (WHC0, whc0_d), (WXC1, wxc1_d), (WHC1, whc1_d),
                    (WCOT, wcoT_d),
                    (B0, b0_d), (B1REP, b1rep_d), (BDOC, bdoc_d),
                    (BC0, bc0_d), (BC1, bc1_d), (BCOC, bcoc_d),
                    (ONES, ones_d), (ID64, id64_d), (IDT, idT_d),
                ]
                for dst, src in loads:
                    sy.dma_start(out=dst[:, :], in_=src[:, :]).then_inc(dw, 16)
                # first subsequence h/c init
                sy.dma_start(out=H0T[:, 0:2 * KT_H * SL], in_=h0T_d[0, :, :]).then_inc(dh, 16)
                sy.dma_start(out=C0SUB[:, 0:256], in_=c0s_d[:, 0:256]).then_inc(dh, 16)

                # conductor: store E_s to DRAM scratch
                for ct in range(CT):
                    sy.wait_ge(dve_s, dve_c(ct, 14))
                    sy.dma_start(out=est_d[ct], in_=EBUF[:, GSL * (ct % 2):GSL * (ct % 2 + 1)]).then_inc(de[ct % 2], 16)

                def n_stores(par):
                    return len([c for c in range(CT) if c % 2 == par])

                # first E prefetch (s=0)
                sy.wait_ge(de[0], 16 * n_stores(0))
                sy.dma_start(out=EDEC[:, 0:GSL], in_=est_d[0]).then_inc(ep[0], 16)

                # decoder phase: per-subsequence prefetch + output DMA
                for t in range(T):
                    s, n = divmod(t, nnotes)
                    if n == 2 and s + 1 < nsub:
                        sy.wait_ge(pe_s, pe_d(t - 1, 5))
                        sp = (s + 1) % 2
                        sy.dma_start(
                            out=H0T[:, sp * (2 * KT_H * SL):(sp + 1) * (2 * KT_H * SL)],
                            in_=h0T_d[s + 1, :, :],
                        ).then_inc(dh, 16)
                        sy.dma_start(out=C0SUB[:, sp * 256:sp * 256 + 256],
                                     in_=c0s_d[:, (s + 1) * 256:(s + 2) * 256]).then_inc(dh, 16)
                        sy.wait_ge(de[sp], 16 * n_stores(sp))
                        sy.dma_start(out=EDEC[:, sp * GSL:(sp + 1) * GSL], in_=est_d[s + 1]).then_inc(ep[sp], 16)
                    p = t % 2
                    sy.wait_ge(act_s, act_d(t, 7))
                    sy.dma_start(out=out_d[t if full_out else 0],
                                 in_=SNT[p][0:64, :].bitcast(F32)).then_inc(do[p], 16)

            # ================= GPSIMD: memsets + exchanges =================
            @block.gpsimd
            def _(g):
                U32 = mybir.dt.uint32
                for tile in (NT[0], NT[1], EMBT[0], EMBT[1]):
                    g.memset(tile[:, :].bitcast(U32), 0).then_inc(gi, 1)
                for tile in (SNT[0], SNT[1], SEM_[0], SEM_[1], HC0[1], HC1[1]):
                    g.memset(tile[:, :].bitcast(U32), 0).then_inc(gi, 1)
                g.memset(CCC0[:, 128:256].bitcast(U32), 0).then_inc(gi, 1)
                g.memset(CCC1[:, 128:256].bitcast(U32), 0).then_inc(gi, 1)
                g.wait_ge(gi, N_MEMSET)
                pid = g.partition_id()
                off = g.scalar_reg_alu(mybir.AluOpType.mult, pid, SL)
                np_ = [0]

                def step_bcasts(specs):
                    # prepare all descriptors first (desc-gen off critical path),
                    # then fire triggers in FIFO order as data becomes ready
                    for stg, gath, rsem, lsem, _, _ in specs:
                        g.remote_dma_broadcast(
                            out_ap=gath[:, bass.ds(off, SL)], in_ap=stg[:, :],
                            remote_sem=rsem, local_sem=lsem, rdests=RD,
                        ).then_inc(prep, 1)
                        np_[0] += 1
                    g.wait_ge(prep, np_[0])
                    for _, _, _, _, wait_sem, wait_val in specs:
                        g.wait_ge(wait_sem, wait_val)
                        g.trigger_dma(count=1)

                for ct in range(CT):
                    p = ct % 2
                    step_bcasts([
                        (HSTG0[p], HC0[p], r_h0, l_h0[p], dve_s, dve_c(ct, 6)),
                        (HSTG1[p], HC1[p], r_h1, l_h1[p], dve_s, dve_c(ct, 12)),
                        (SEM_[p], EMBT[p], r_em, l_em[p], act_s, act_c(ct, 7)),
                    ])
                for t in range(T):
                    p = t % 2
                    step_bcasts([
                        (HSTG0[p], HD0[p], r_h0, l_h0[p], dve_s, dve_d(t, 6)),
                        (HSTG1[p], HD1[p], r_h1, l_h1[p], dve_s, dve_d(t, 12)),
                        (SNT[p], NT[p], r_nt, l_nt[p], act_s, act_d(t, 7)),
                    ])

            # ================= TENSOR: matmuls + transposes =================
            @block.tensor
            def _(t_):
                # Gate-column split: every K-tile issues TWO N=256 matmuls that
                # run concurrently in array col-groups 0-63 / 64-127:
                #   lo = gate cols 0:256  ([i|f]) -> psum partitions 0:64
                #   hi = gate cols 256:512 ([g|o]) -> psum partitions 64:128
                class Grp:
                    def __init__(self, bank):
                        self.bank = bank
                        self.started = [False, False]

                    def mm(self, h, lhsT, rhs, stop=False, inc=None):
                        m = t_.matmul(
                            self.bank[64 * h:64 * h + 64, :], lhsT, rhs,
                            start=not self.started[h], stop=stop,
                            skip_group_check=True)
                        self.started[h] = True
                        if inc is not None:
                            m.then_inc(inc, 1)
                        return m

                def split_run(grp, tiles, inc=None, final=False):
                    for i, (lhsT, rlo, rhi) in enumerate(tiles):
                        last = final and i == len(tiles) - 1
                        grp.mm(0, lhsT, rlo, stop=last)
                        grp.mm(1, lhsT, rhi, stop=last,
                               inc=inc if last else None)

                def tl(lhsT, W, base, prange=None):
                    if prange is None:
                        return (lhsT, W[:, base:base + 256],
                                W[:, base + 256:base + 512])
                    s_, e_ = prange
                    return (lhsT, W[s_:e_, base:base + 256],
                            W[s_:e_, base + 256:base + 512])

                t_.wait_ge(dw, 16 * N_WLOAD)
                t_.wait_ge(gi, N_MEMSET)

                # ---------- conductor ----------
                for ct in range(CT):
                    p, p1 = ct % 2, (ct - 1) % 2
                    g0 = Grp(psd0[p])
                    g1 = Grp(psd1[p])
                    if ct >= 2:
                        t_.wait_ge(act_s, act_c(ct - 2, 2))   # psd0 last reader
                        t_.wait_ge(act_s, act_c(ct - 2, 5))   # psd1 last reader
                    split_run(g0, [tl(ONES[:, :], BC0, 0)])
                    split_run(g1, [tl(ONES[:, :], BC1, 0)])
                    lat = [tl(LAT[:, SL * k:SL * (k + 1)], WXC0, GSL * k)
                           for k in range(KT_L)]
                    split_run(g0, lat)
                    if ct >= 1:
                        t_.wait_ge(r_h0, 16 * ct)
                    hc_t = [tl(HC0[p1][:, SL * k:SL * (k + 1)], WHC0, GSL * k)
                            for k in range(KT_H)]
                    split_run(g0, hc_t, inc=pe_s, final=True)
                    t_.wait_ge(dve_s, dve_c(ct, 5))
                    t_.transpose(pstr0[:, 0:32].bitcast(BF16), HT0[:, :], IDT[0:64, 0:64]).then_inc(pe_s, 1)
                    hp = [tl(HC1[p1][:, SL * k:SL * (k + 1)], WHC1, GSL * k)
                          for k in range(KT_H)]
                    split_run(g1, hp)
                    t_.wait_ge(r_h0, 16 * (ct + 1))
                    hx = [tl(HC0[p][:, SL * k:SL * (k + 1)], WXC1, GSL * k)
                          for k in range(KT_H)]
                    split_run(g1, hx, inc=pe_s, final=True)
                    t_.wait_ge(dve_s, dve_c(ct, 11))
                    t_.transpose(pstr1[:, 0:32].bitcast(BF16), HT1[:, :], IDT[0:64, 0:64]).then_inc(pe_s, 1)
                    # emb projection (serial N=64 bf16 group, cols 0-63)
                    if ct >= 1:
                        t_.wait_ge(act_s, act_c(ct - 1, 7))
                    t_.wait_ge(r_h1, 16 * (ct + 1))
                    for k in range(KT_H):
                        m = t_.matmul(pspr[0:64, :], WCOT[:, SL * k:SL * (k + 1)],
                                      HC1[p][:, SL * k:SL * (k + 1)],
                                      start=(k == 0), stop=(k == KT_H - 1),
                                      skip_group_check=True)
                    m.then_inc(pe_s, 1)
                    # E_s = b0 + emb@Wemb-part  (uses gathered EMBT)
                    if ct >= 2:
                        t_.wait_ge(dve_s, dve_c(ct - 2, 14))
                    ge = Grp(psem)
                    split_run(ge, [tl(ONES[:, :], B0, 0)])
                    t_.wait_ge(r_em, 16 * (ct + 1))
                    em = [tl(EMBT[p][0:64, SL * j:SL * (j + 1)], WEMB, GSL * j,
                             prange=(0, 64)) for j in range(8)]
                    split_run(ge, em, inc=pe_s, final=True)

                # ---------- decoder ----------
                for t in range(T):
                    p, p1 = t % 2, (t - 1) % 2
                    s, n = divmod(t, nnotes)
                    sb_ = s % 2
                    h0base = sb_ * (2 * KT_H * SL)
                    g0 = Grp(psd0[p])
                    g1 = Grp(psd1[p])
                    if t >= 2:
                        t_.wait_ge(act_s, act_d(t - 2, 2))
                        t_.wait_ge(act_s, act_d(t - 2, 5))
                    else:
                        t_.wait_ge(act_s, act_c(CT - 2 + t, 2))
                        t_.wait_ge(act_s, act_c(CT - 2 + t, 5))
                    if n == 0:
                        t_.wait_ge(ep[s % 2], 16 * ((s - s % 2) // 2 + 1))
                    split_run(g0, [tl(ID64[:, :], EDEC, GSL * (s % 2), prange=(0, 64))])
                    split_run(g1, [tl(ID64[:, :], B1REP, 0)])
                    # ---- layer d0 ----
                    if n == 0:
                        t_.wait_ge(dh, 32 * (s + 1))
                        stat = lambda k: H0T[:, h0base + SL * k:h0base + SL * (k + 1)]
                    else:
                        stat = lambda k: HD0[p1][:, SL * k:SL * (k + 1)]
                    h0t = [tl(stat(k), WH0, GSL * k) for k in range(KT_H)]
                    split_run(g0, h0t)
                    if t >= 1:
                        t_.wait_ge(r_nt, 16 * t)
                    nt_ = [tl(NT[p1][0:64, SL * j:SL * (j + 1)], WX0, GSL * j,
                              prange=(0, 64)) for j in range(8)]
                    split_run(g0, nt_, inc=pe_s, final=True)
                    t_.wait_ge(dve_s, dve_d(t, 5))
                    t_.transpose(pstr0[:, 0:32].bitcast(BF16), HT0[:, :], IDT[0:64, 0:64]).then_inc(pe_s, 1)
                    # ---- layer d1 ----
                    if n == 0:
                        stat1 = lambda k: H0T[:, h0base + (KT_H + k) * SL:h0base + (KT_H + k + 1) * SL]
                    else:
                        stat1 = lambda k: HD1[p1][:, SL * k:SL * (k + 1)]
                    h1t = [tl(stat1(k), WH1, GSL * k) for k in range(KT_H)]
                    split_run(g1, h1t)
                    t_.wait_ge(r_h0, 16 * (CT + t + 1))
                    wxt = [tl(HD0[p][:, SL * k:SL * (k + 1)], WX1, GSL * k)
                           for k in range(KT_H)]
                    split_run(g1, wxt, inc=pe_s, final=True)
                    t_.wait_ge(dve_s, dve_d(t, 11))
                    t_.transpose(pstr1[:, 0:32].bitcast(BF16), HT1[:, :], IDT[0:64, 0:64]).then_inc(pe_s, 1)
                    # ---- note projection (serial N=64 bf16 group) ----
                    if t >= 1:
                        t_.wait_ge(act_s, act_d(t - 1, 7))
                    else:
                        t_.wait_ge(act_s, act_c(CT - 1, 7))
                    t_.wait_ge(r_h1, 16 * (CT + t + 1))
                    for k in range(KT_H):
                        m = t_.matmul(pspr[0:64, :], WDOT[:, SL * k:SL * (k + 1)],
                                      HD1[p][:, SL * k:SL * (k + 1)],
                                      start=(k == 0), stop=(k == KT_H - 1),
                                      skip_group_check=True)
                    m.then_inc(pe_s, 1)

            # ================= SCALAR (ACT) =================
            @block.scalar
            def _(a):
                SIG = mybir.ActivationFunctionType.Sigmoid
                TANH = mybir.ActivationFunctionType.Tanh

                def layer_acts(pe_done, dve_cn, dve_hprev, S, CC, TTt, psrc):
                    a.wait_ge(pe_s, pe_done)
                    if dve_hprev is not None:
                        a.wait_ge(dve_s, dve_hprev)
                    # hi half ([g|o], psum partitions 64:128) first: g' heads
                    # the DVE chain
                    a.activation(S[:, 256:512], psrc[64:128, :], SIG).then_inc(act_s, 1)
                    a.activation(S[:, 0:256], psrc[0:64, :], SIG).then_inc(act_s, 1)
                    a.wait_ge(dve_s, dve_cn)
                    a.activation(TTt[:, :], CC[:, 128:256], TANH).then_inc(act_s, 1)

                # conductor
                for ct in range(CT):
                    p = ct % 2
                    layer_acts(pe_c(ct, 1), dve_c(ct, 4),
                               dve_c(ct - 1, 5) if ct >= 1 else None,
                               S0, CCC0, TT0, psd0[p])
                    layer_acts(pe_c(ct, 3), dve_c(ct, 10),
                               dve_c(ct - 1, 11) if ct >= 1 else None,
                               S1, CCC1, TT1, psd1[p])
                    a.wait_ge(pe_s, pe_c(ct, 5))
                    if snd_c(ct) > 0:
                        a.wait_ge(l_em[ct % 2], 16 * snd_c(ct))
                    a.activation(SEM_[p][0:64, :], pspr[0:64, :], TANH,
                                 bias=BCOC[:, 0:1]).then_inc(act_s, 1)
                # decoder
                for t in range(T):
                    p = t % 2
                    layer_acts(pe_d(t, 1), dve_d(t, 4),
                               dve_d(t - 1, 5) if t >= 1 else dve_c(CT - 1, 5),
                               S0, CC0, TT0, psd0[p])
                    layer_acts(pe_d(t, 3), dve_d(t, 10),
                               dve_d(t - 1, 11) if t >= 1 else dve_c(CT - 1, 11),
                               S1, CC1, TT1, psd1[p])
                    a.wait_ge(pe_s, pe_d(t, 5))
                    if t >= 2:
                        a.wait_ge(l_nt[t % 2], 16 * ((t - t % 2) // 2))
                        a.wait_ge(do[t % 2], 16 * ((t - t % 2) // 2))
                    a.activation(SNT[p][0:64, :], pspr[0:64, :], TANH,
                                 bias=BDOC[:, 0:1]).then_inc(act_s, 1)

            # ================= VECTOR (DVE) =================
            @block.vector
            def _(v):
                MUL = mybir.AluOpType.mult
                ADD = mybir.AluOpType.add
                SUB = mybir.AluOpType.subtract

                def layer_chain(base, sigH_done, sigL_done, tanh_done, tr_done,
                                l_sem, l_val, S, CC, TMP, TTt, HTt, pstr,
                                HSTGt, c_src):
                    # g' = 2*sig(2g) - 1
                    v.wait_ge(act_s, sigH_done)
                    v.tensor_scalar(CC[:, 0:128], S[:, 256:384], 2.0, 1.0, MUL, SUB).then_inc(dve_s, 1)
                    # tmp_i = S_i * g' ; tmp_f = S_f * c
                    v.wait_ge(act_s, sigL_done)
                    v.wait_ge(dve_s, base + 1)
                    v.tensor_tensor(TMP[:, 0:128], S[:, 0:128], CC[:, 0:128], MUL).then_inc(dve_s, 1)
                    v.tensor_tensor(TMP[:, 128:256], S[:, 128:256], c_src, MUL).then_inc(dve_s, 1)
                    # c_new
                    v.wait_ge(dve_s, base + 3)
                    v.tensor_tensor(CC[:, 128:256], TMP[:, 0:128], TMP[:, 128:256], ADD).then_inc(dve_s, 1)
                    # h = S_o * tanh(c)
                    v.wait_ge(act_s, tanh_done)
                    v.tensor_tensor(HTt[:, :], S[:, 384:512], TTt[:, :], MUL).then_inc(dve_s, 1)
                    # copy transpose psum -> staging (bf16)
                    v.wait_ge(pe_s, tr_done)
                    if l_val > 0:
                        v.wait_ge(l_sem, l_val)
                    v.tensor_copy(HSTGt[:, :], pstr[:, 0:32].bitcast(BF16)).then_inc(dve_s, 1)

                v.wait_ge(gi, N_MEMSET)
                for ct in range(CT):
                    p = ct % 2
                    layer_chain(dve_c(ct, 0), act_c(ct, 1), act_c(ct, 2), act_c(ct, 3), pe_c(ct, 2),
                                l_h0[p], 16 * snd_c(ct), S0, CCC0, TMP0, TT0, HT0,
                                pstr0, HSTG0[p], CCC0[:, 128:256])
                    layer_chain(dve_c(ct, 6), act_c(ct, 4), act_c(ct, 5), act_c(ct, 6), pe_c(ct, 4),
                                l_h1[p], 16 * snd_c(ct), S1, CCC1, TMP1, TT1, HT1,
                                pstr1, HSTG1[p], CCC1[:, 128:256])
                    # E halves -> EBUF (bf16; DRAM-bounced by sync)
                    v.wait_ge(pe_s, pe_c(ct, 6))
                    if (ct - ct % 2) // 2 > 0:
                        v.wait_ge(de[ct % 2], 16 * ((ct - ct % 2) // 2))
                    v.tensor_copy(EBUF[:, GSL * (ct % 2):GSL * (ct % 2) + 256],
                                  psem[0:64, :]).then_inc(dve_s, 1)
                    v.tensor_copy(EBUF[:, GSL * (ct % 2) + 256:GSL * (ct % 2 + 1)],
                                  psem[64:128, :]).then_inc(dve_s, 1)
                # decoder
                for t in range(T):
                    p = t % 2
                    s, n = divmod(t, nnotes)
                    sp_ = s % 2
                    c0src = C0SUB[:, sp_ * 256:sp_ * 256 + 128] if n == 0 else CC0[:, 128:256]
                    c1src = C0SUB[:, sp_ * 256 + 128:sp_ * 256 + 256] if n == 0 else CC1[:, 128:256]
                    if n == 0:
                        v.wait_ge(dh, 32 * (s + 1))
                    layer_chain(dve_d(t, 0), act_d(t, 1), act_d(t, 2), act_d(t, 3), pe_d(t, 2),
                                l_h0[p], 16 * snd_d(t), S0, CC0, TMP0, TT0, HT0,
                                pstr0, HSTG0[p], c0src)
                    layer_chain(dve_d(t, 6), act_d(t, 4), act_d(t, 5), act_d(t, 6), pe_d(t, 4),
                                l_h1[p], 16 * snd_d(t), S1, CC1, TMP1, TT1, HT1,
                                pstr1, HSTG1[p], c1src)

    nc.compile()
    return nc


# ======================= host-side preparation =======================

def _gate_slice_ixs(core):
    """Column indices (into the 4H gate dim, PyTorch i,f,g,o order) for one
    core's 512-gate slice, ordered [i(128) f(128) g(128) o(128)]."""
    ix = []
    for gg in range(4):
        base = gg * H + core * 128
        ix.extend(range(base, base + 128))
    return np.array(ix)


def prep_inputs(inputs, nsub=16, nnotes=32):
    f = lambda x: np.asarray(x, dtype=np.float32)
    latent = f(inputs["latent"])
    h0_dec = f(inputs["h0_dec"])[:nsub]
    c0_dec = f(inputs["c0_dec"])[:nsub]

    def pack_k(wT, kt):
        # wT: [K, N] -> [128, kt*N] tiles along K
        K, N = wT.shape
        assert K == kt * 128
        out = np.empty((128, kt * N), np.float32)
        for k in range(kt):
            out[:, N * k:N * (k + 1)] = wT[128 * k:128 * (k + 1), :]
        return out

    def pack_k64(wT, kt):
        K, N = wT.shape
        assert K == kt * 64
        out = np.empty((64, kt * N), np.float32)
        for k in range(kt):
            out[:, N * k:N * (k + 1)] = wT[64 * k:64 * (k + 1), :]
        return out

    # h0T packed: [s, p, (l k b)]
    h0T = np.einsum("slbk->slkb", h0_dec)  # [s, l, 1024, 64]
    h0T_packed = np.empty((nsub, 128, 2 * KT_H * SL), np.float32)
    for s in range(nsub):
        for l in range(2):
            for k in range(KT_H):
                h0T_packed[s, :, (l * KT_H + k) * SL:(l * KT_H + k + 1) * SL] = \
                    h0T[s, l, 128 * k:128 * (k + 1), :]

    latT = np.ascontiguousarray(latent.T)  # [512, 64]
    latT_packed = pack_k(latT, KT_L)

    ident64 = np.eye(64, dtype=np.float32)
    identT = np.eye(128, dtype=np.float32)
    ones_row = np.ones((1, SL), np.float32)

    Wih_d0, Whh_d0 = f(inputs["Wih_d0"]), f(inputs["Whh_d0"])
    Wih_d1, Whh_d1 = f(inputs["Wih_d1"]), f(inputs["Whh_d1"])
    Wdo, bdo = f(inputs["Wdo"]), f(inputs["bdo"])
    Wih_c0, Whh_c0 = f(inputs["Wih_c0"]), f(inputs["Whh_c0"])
    Wih_c1, Whh_c1 = f(inputs["Wih_c1"]), f(inputs["Whh_c1"])
    Wco, bco = f(inputs["Wco"]), f(inputs["bco"])
    b0_full = f(inputs["bih_d0"]) + f(inputs["bhh_d0"])
    b1_full = f(inputs["bih_d1"]) + f(inputs["bhh_d1"])
    bc0_full = f(inputs["bih_c0"]) + f(inputs["bhh_c0"])
    bc1_full = f(inputs["bih_c1"]) + f(inputs["bhh_c1"])

    Wdo_pad = np.zeros((INPUT_PAD, H), np.float32)
    Wdo_pad[:INPUT] = Wdo
    bdo_pad = np.zeros(INPUT_PAD, np.float32)
    bdo_pad[:INPUT] = bdo

    in_maps = []
    for core in range(NC):
        ix = _gate_slice_ixs(core)
        gmask = np.ones(GSL, np.float32)
        gmask[256:384] = 2.0  # double g-gate pre-activations

        def slc(w, xdim=None):
            # w: [4H, K] -> [K, 512] slice with g-doubling
            wT = w[ix, :].T.astype(np.float32) * gmask[None, :]
            return np.ascontiguousarray(wT)

        wx0_full = np.zeros((INPUT_PAD, GSL), np.float32)
        wx0_full[:INPUT] = slc(Wih_d0[:, :INPUT])
        wemb_full = slc(Wih_d0[:, INPUT:INPUT + COND_OUT])  # [512, 512]

        b1r = (b1_full[ix] * gmask)
        m = {
            "latT": latT_packed,
            "h0T": h0T_packed,
            "c0s": np.ascontiguousarray(
                c0_dec[:, :, :, core * 128:(core + 1) * 128].transpose(2, 0, 1, 3).reshape(B, -1)),
            "wx0": pack_k64(wx0_full, 8),
            "wh0": pack_k(slc(Whh_d0), KT_H),
            "wx1": pack_k(slc(Wih_d1), KT_H),
            "wh1": pack_k(slc(Whh_d1), KT_H),
            "wdoT": pack_k(np.ascontiguousarray(Wdo_pad.T[:, core * SL:(core + 1) * SL]), KT_H),
            "wemb": pack_k64(wemb_full, 8),
            "wxc0": pack_k(slc(Wih_c0), KT_L),
            "whc0": pack_k(slc(Whh_c0), KT_H),
            "wxc1": pack_k(slc(Wih_c1), KT_H),
            "whc1": pack_k(slc(Whh_c1), KT_H),
            "wcoT": pack_k(np.ascontiguousarray(Wco.T[:, core * SL:(core + 1) * SL]), KT_H),
            "b0r": (b0_full[ix] * gmask)[None, :],
            "b1rep": np.tile(b1r[None, :], (64, 1)),
            "bdoc": bdo_pad[core * SL:(core + 1) * SL][:, None],
            "bc0r": (bc0_full[ix] * gmask)[None, :],
            "bc1r": (bc1_full[ix] * gmask)[None, :],
            "bcoc": bco[core * SL:(core + 1) * SL][:, None],
            "onesr": ones_row,
            "id64": ident64,
            "idT": identT,
        }
        import ml_dtypes
        F32_KEYS = {"c0s", "bdoc", "bcoc"}
        in_maps.append({
            k: np.ascontiguousarray(
                v, dtype=(np.float32 if k in F32_KEYS else ml_dtypes.bfloat16))
            for k, v in m.items()
        })
    return in_maps


def assemble_output(results, nsub=16, nnotes=32):
    T = nsub * nnotes
    # each core: out [T, 64(note rows), 64(batch)] bf16 -> concat note rows
    full = np.concatenate(
        [np.asarray(results[c]["out"], dtype=np.float32) for c in range(NC)],
        axis=1)  # [T, 512, 64]
    return np.ascontiguousarray(full[:, :INPUT, :].transpose(2, 0, 1))  # [B, T, INPUT]


_CACHED = {}


def kernel(**inputs) -> np.ndarray:
    from concourse.bass_utils import run_bass_kernel_spmd
    nsub, nnotes = 16, 32
    key = (nsub, nnotes)
    if key not in _CACHED:
        _CACHED[key] = build(nsub, nnotes)
    nc = _CACHED[key]
    in_maps = prep_inputs(inputs, nsub, nnotes)
    res = run_bass_kernel_spmd(nc, in_maps, core_ids=list(range(NC)))
    return assemble_output(res.results, nsub, nnotes)
